# revision 1
# baseline (speedup 1.0000x reference)
"""MultiHeadAttention forward on 8 Trainium2 NeuronCores.

Problem: B=2, S=2048, D_MODEL=1024, H=16 heads, d_k=64, causal mask.

Sharding: core c in [0,8) owns heads {2c, 2c+1} for BOTH batches.
 - Projections: each core computes Q^T,K^T ([128=2*d_k, B*S]) and V
   ([B*S, 2*65]) for its 2 heads (contraction over full d_model).
 - Attention in the "transposed scores" orientation: scoresT[kpos, q] =
   K^T.T @ Q^T per head, exp on ScalarE (scale=1/8 folded in, no max
   subtraction -- scores are O(+-6) so exp is safe in fp32), causal mask
   applied by multiplying the (at most 4) diagonal tiles with
   precomputed 0/1 masks. attn_outT[dv, q] accumulates via matmuls with
   V tiles as the stationary operand; a ones-column appended to V yields
   the softmax denominators in the same matmul.
 - Normalization: reciprocal of the sums row, partition-broadcast,
   multiply while copying psum -> the bf16 attn_flatT tile [128, B*S].
 - AllToAll over all 8 cores redistributes attn_flatT so core j ends up
   with all 1024 d_model rows for its output slice (batch j//4, rows
   512*(j%4) .. +512), then out = attn_flat @ W_o + b_o locally.

Matmuls run in bf16 (inputs are cast on the host); accumulation is fp32
in PSUM. Host-side numpy simulation of this scheme gives ~5.6e-3
norm-relative error vs the fp32 reference.
"""

import sys

import numpy as np

sys.path.insert(0, "/opt/trn_rl_repo")

import ml_dtypes  # noqa: E402

import concourse.bacc as bacc  # noqa: E402
import concourse.mybir as mybir  # noqa: E402
import concourse.tile as tile  # noqa: E402
from concourse.bass_utils import run_bass_kernel_spmd  # noqa: E402

F32 = mybir.dt.float32
BF16 = mybir.dt.bfloat16
BF = ml_dtypes.bfloat16

B, S, D, H, DK = 2, 2048, 1024, 16, 64
N_CORES = 8
BS = B * S  # 4096
HPC = H // N_CORES  # heads per core = 2
DPC = HPC * DK  # d_model slice per core = 128
S_OUT = S // 4  # output rows per core = 512

_CACHED = {}


def build_nc():
    nc = bacc.Bacc(num_devices=N_CORES)

    # ---- I/O ----
    xq = nc.dram_tensor("xq", [D, BS], BF16, kind="ExternalInput")  # q[b].T concat
    xk = nc.dram_tensor("xk", [D, BS], BF16, kind="ExternalInput")
    xv = nc.dram_tensor("xv", [D, BS], BF16, kind="ExternalInput")
    wq = nc.dram_tensor("wq", [D, DPC], BF16, kind="ExternalInput")  # W_q[:, my cols]
    wk = nc.dram_tensor("wk", [D, DPC], BF16, kind="ExternalInput")
    wv = nc.dram_tensor("wv", [D, DPC], BF16, kind="ExternalInput")
    wo = nc.dram_tensor("wo", [D, D], BF16, kind="ExternalInput")  # full W_o
    bq = nc.dram_tensor("bq", [DPC, 1], F32, kind="ExternalInput")
    bk = nc.dram_tensor("bk", [DPC, 1], F32, kind="ExternalInput")
    bv = nc.dram_tensor("bv", [1, DPC], F32, kind="ExternalInput")
    bo = nc.dram_tensor("bo", [1, D], F32, kind="ExternalInput")
    masks = nc.dram_tensor("masks", [128, 2048], BF16, kind="ExternalInput")
    out = nc.dram_tensor("out", [S_OUT, D], F32, kind="ExternalOutput")

    NKT = S // 128  # kpos tiles per batch = 16
    NQB = S // 512  # q blocks per batch = 4

    with tile.TileContext(nc) as tc:
        with (
            tc.tile_pool(name="xtq", bufs=8) as xq_pool,
            tc.tile_pool(name="xtk", bufs=8) as xk_pool,
            tc.tile_pool(name="xtv", bufs=8) as xv_pool,
            tc.tile_pool(name="wtiles", bufs=1) as w_pool,
            tc.tile_pool(name="persist", bufs=1) as persist,
            tc.tile_pool(name="exp", bufs=4) as exp_pool,
            tc.tile_pool(name="outsb", bufs=2) as out_pool,
            tc.tile_pool(name="small", bufs=2) as small_pool,
            tc.tile_pool(name="gen_ps", bufs=2, space="PSUM") as gen_ps,
            tc.tile_pool(name="score_ps", bufs=2, space="PSUM") as score_ps,
            tc.tile_pool(name="av_ps", bufs=1, space="PSUM") as av_ps,
            tc.tile_pool(name="dram", bufs=1, space="DRAM") as dram,
        ):
            # ---- persistent SBUF tensors ----
            QT = persist.tile([128, BS], BF16, tag="QT")  # rows: hA d 0-63, hB 64-127
            KT = persist.tile([128, BS], BF16, tag="KT")
            VA = [persist.tile([128, DK + 1], BF16, tag=f"VA{i}", name=f"VA{i}") for i in range(2 * NKT)]
            VB = [persist.tile([128, DK + 1], BF16, tag=f"VB{i}", name=f"VB{i}") for i in range(2 * NKT)]
            AFT = persist.tile([128, BS], BF16, tag="AFT")  # attn_flatT
            mask_t = persist.tile([128, 2048], BF16, tag="mask")
            nc.sync.dma_start(mask_t[:], masks[:])

            wq_t = [w_pool.tile([128, DPC], BF16, tag=f"wq{d}", name=f"wq{d}") for d in range(8)]
            wk_t = [w_pool.tile([128, DPC], BF16, tag=f"wk{d}", name=f"wk{d}") for d in range(8)]
            wv_t = [w_pool.tile([128, DPC], BF16, tag=f"wv{d}", name=f"wv{d}") for d in range(8)]
            wo_t = [w_pool.tile([128, D], BF16, tag=f"wo{d}", name=f"wo{d}") for d in range(8)]
            for d in range(8):
                nc.sync.dma_start(wq_t[d][:], wq[128 * d : 128 * (d + 1), :])
                nc.sync.dma_start(wk_t[d][:], wk[128 * d : 128 * (d + 1), :])
                nc.sync.dma_start(wv_t[d][:], wv[128 * d : 128 * (d + 1), :])
                nc.sync.dma_start(wo_t[d][:], wo[128 * d : 128 * (d + 1), :])

            bq_t = persist.tile([DPC, 1], F32, tag="bq")
            bk_t = persist.tile([DPC, 1], F32, tag="bk")
            nc.sync.dma_start(bq_t[:], bq[:])
            nc.sync.dma_start(bk_t[:], bk[:])
            bv_bc = persist.tile([128, DPC], F32, tag="bvbc")
            nc.sync.dma_start(bv_bc[:], bv[:].partition_broadcast(128))
            bo_bc = persist.tile([128, D], F32, tag="bobc")
            nc.sync.dma_start(bo_bc[:], bo[:].partition_broadcast(128))

            # ---- projections, then attention, per batch ----
            for b in range(B):
                scol = S * b
                # load x^T tiles for this batch (all of q,k,v)
                xq_t = [xq_pool.tile([128, S], BF16, tag="xtq", name="xtq") for _ in range(8)]
                xk_t = [xk_pool.tile([128, S], BF16, tag="xtk", name="xtk") for _ in range(8)]
                xv_t = [xv_pool.tile([128, S], BF16, tag="xtv", name="xtv") for _ in range(8)]
                for d in range(8):
                    dsl = slice(128 * d, 128 * (d + 1))
                    nc.sync.dma_start(xq_t[d][:], xq[dsl, scol : scol + S])
                    nc.sync.dma_start(xk_t[d][:], xk[dsl, scol : scol + S])
                    nc.sync.dma_start(xv_t[d][:], xv[dsl, scol : scol + S])

                # Q^T and K^T projections: psum[dout 128, s 512]
                for name, xt_, wt_, bias in (
                    ("q", xq_t, wq_t, bq_t),
                    ("k", xk_t, wk_t, bk_t),
                ):
                    dst = QT if name == "q" else KT
                    for sc in range(4):
                        ps = gen_ps.tile([128, 512], F32, tag="gen")
                        for d in range(8):
                            nc.tensor.matmul(
                                ps[:],
                                wt_[d][:],
                                xt_[d][:, 512 * sc : 512 * (sc + 1)],
                                start=(d == 0),
                                stop=(d == 7),
                            )
                        nc.vector.tensor_scalar_add(
                            dst[:, scol + 512 * sc : scol + 512 * (sc + 1)],
                            ps[:],
                            bias[:],
                        )

                # V projection: psum[s 128, dv 128] -> VA/VB tiles [128, 65]
                for ss in range(NKT):
                    ps = gen_ps.tile([128, 128], F32, tag="gen")
                    for d in range(8):
                        nc.tensor.matmul(
                            ps[:],
                            xv_t[d][:, 128 * ss : 128 * (ss + 1)],
                            wv_t[d][:],
                            start=(d == 0),
                            stop=(d == 7),
                        )
                    va = VA[NKT * b + ss]
                    vb = VB[NKT * b + ss]
                    nc.vector.tensor_add(va[:, 0:DK], ps[:, 0:DK], bv_bc[:, 0:DK])
                    nc.vector.tensor_add(vb[:, 0:DK], ps[:, DK : 2 * DK], bv_bc[:, DK : 2 * DK])
                    nc.vector.memset(va[:, DK : DK + 1], 1.0)
                    nc.vector.memset(vb[:, DK : DK + 1], 1.0)

                # ---- attention for this batch ----
                def emit_scores(kt, qsl):
                    # scoresT for both heads (row-packed, d_k=64 each)
                    ksl = slice(scol + 128 * kt, scol + 128 * (kt + 1))
                    ps = score_ps.tile([128, 1024], F32, tag="sc", name="sc")
                    nc.tensor.matmul(
                        ps[:, 0:512], KT[0:64, ksl], QT[0:64, qsl],
                        start=True, stop=True,
                    )
                    nc.tensor.matmul(
                        ps[:, 512:1024], KT[64:128, ksl], QT[64:128, qsl],
                        start=True, stop=True,
                    )
                    return ps

                for qb in range(NQB):
                    qsl = slice(scol + 512 * qb, scol + 512 * (qb + 1))
                    n_kt = 4 * qb + 4
                    av_a = av_ps.tile([DK + 1, 512], F32, tag="av_a")
                    av_b = av_ps.tile([DK + 1, 512], F32, tag="av_b")
                    # software pipeline: scores(kt+1) is emitted before
                    # attnV(kt) so PE fills the exp(kt) latency with the
                    # next tile's score matmuls (score_ps bufs=2).
                    ps_cur = emit_scores(0, qsl)
                    for kt in range(n_kt):
                        et = exp_pool.tile([128, 1024], BF16, tag="et")
                        nc.scalar.activation(
                            et[:], ps_cur[:], mybir.ActivationFunctionType.Exp,
                            scale=0.125,
                        )
                        if kt + 1 < n_kt:
                            ps_cur = emit_scores(kt + 1, qsl)
                        t = kt - 4 * qb
                        if t >= 0:
                            msl = slice(512 * t, 512 * (t + 1))
                            nc.vector.tensor_mul(et[:, 0:512], et[:, 0:512], mask_t[:, msl])
                            nc.vector.tensor_mul(et[:, 512:1024], et[:, 512:1024], mask_t[:, msl])
                        nc.tensor.matmul(
                            av_a[:], VA[NKT * b + kt][:], et[:, 0:512],
                            start=(kt == 0), stop=(kt == n_kt - 1),
                        )
                        nc.tensor.matmul(
                            av_b[:], VB[NKT * b + kt][:], et[:, 512:1024],
                            start=(kt == 0), stop=(kt == n_kt - 1),
                        )
                    # copy psum out fast (frees the attnV banks for the next
                    # q-block), then normalize off the critical path
                    for av, row0 in ((av_a, 0), (av_b, 64)):
                        avs = small_pool.tile([DK + 1, 512], F32, tag="avs", name="avs")
                        nc.vector.tensor_copy(avs[:], av[:])
                        rc = small_pool.tile([1, 512], F32, tag="recip")
                        nc.vector.reciprocal(rc[:], avs[DK : DK + 1, :])
                        rbc = small_pool.tile([64, 512], F32, tag="rbc")
                        nc.gpsimd.partition_broadcast(rbc[:], rc[:])
                        nc.vector.tensor_mul(AFT[row0 : row0 + 64, qsl], avs[0:DK, :], rbc[:])

            # ---- AllToAll: redistribute attn_flatT ----
            a2a_in = dram.tile([N_CORES * 128, 512], BF16, tag="a2a_in")
            a2a_out = dram.tile([N_CORES * 128, 512], BF16, tag="a2a_out")
            for j in range(N_CORES):
                nc.sync.dma_start(
                    a2a_in[128 * j : 128 * (j + 1), :],
                    AFT[:, 512 * j : 512 * (j + 1)],
                )
            nc.gpsimd.collective_compute(
                "AllToAll",
                mybir.AluOpType.bypass,
                replica_groups=[list(range(N_CORES))],
                ins=[a2a_in[:]],
                outs=[a2a_out[:]],
            )
            lhs_t = [persist.tile([128, 512], BF16, tag=f"lhs{i}", name=f"lhs{i}") for i in range(8)]
            for i in range(8):
                nc.sync.dma_start(lhs_t[i][:], a2a_out[128 * i : 128 * (i + 1), :])

            # ---- W_o matmul for my 512 output rows ----
            for st in range(4):
                osb = out_pool.tile([128, D], F32, tag="osb")
                for nch in range(2):
                    ps = gen_ps.tile([128, 512], F32, tag="gen")
                    for i in range(8):
                        nc.tensor.matmul(
                            ps[:],
                            lhs_t[i][:, 128 * st : 128 * (st + 1)],
                            wo_t[i][:, 512 * nch : 512 * (nch + 1)],
                            start=(i == 0),
                            stop=(i == 7),
                        )
                    nc.vector.tensor_add(
                        osb[:, 512 * nch : 512 * (nch + 1)],
                        ps[:],
                        bo_bc[:, 512 * nch : 512 * (nch + 1)],
                    )
                nc.sync.dma_start(out[128 * st : 128 * (st + 1), :], osb[:])

    nc.finalize()
    return nc


def _prep_in_maps(q, k, v, W_q, b_q, W_k, b_k, W_v, b_v, W_o, b_o):
    def xT(x):  # [B,S,D] f32 -> [D, B*S] bf16
        return np.ascontiguousarray(
            x.reshape(BS, D).T.astype(BF)
        )

    xq_h, xk_h, xv_h = xT(q), xT(k), xT(v)
    wo_h = np.ascontiguousarray(W_o.astype(BF))
    bo_h = np.ascontiguousarray(b_o.reshape(1, D).astype(np.float32))

    # masks: mask_t[i, 512*t + j] = 1 if 128*t + i <= j else 0
    i = np.arange(128)[:, None]
    j = np.arange(512)[None, :]
    masks_h = np.concatenate(
        [(128 * t + i <= j) for t in range(4)], axis=1
    ).astype(BF)

    in_maps = []
    for c in range(N_CORES):
        csl = slice(DPC * c, DPC * (c + 1))
        in_maps.append(
            {
                "xq": xq_h,
                "xk": xk_h,
                "xv": xv_h,
                "wq": np.ascontiguousarray(W_q[:, csl].astype(BF)),
                "wk": np.ascontiguousarray(W_k[:, csl].astype(BF)),
                "wv": np.ascontiguousarray(W_v[:, csl].astype(BF)),
                "wo": wo_h,
                "bq": np.ascontiguousarray(
                    b_q[csl].reshape(DPC, 1).astype(np.float32)
                ),
                "bk": np.ascontiguousarray(
                    b_k[csl].reshape(DPC, 1).astype(np.float32)
                ),
                "bv": np.ascontiguousarray(
                    b_v[csl].reshape(1, DPC).astype(np.float32)
                ),
                "bo": bo_h,
                "masks": masks_h,
            }
        )
    return in_maps


def kernel(q, k, v, mask, W_q, b_q, W_k, b_k, W_v, b_v, W_o, b_o, **run_kwargs):
    q, k, v = (np.asarray(t, np.float32) for t in (q, k, v))
    in_maps = _prep_in_maps(
        q, k, v,
        np.asarray(W_q, np.float32), np.asarray(b_q, np.float32),
        np.asarray(W_k, np.float32), np.asarray(b_k, np.float32),
        np.asarray(W_v, np.float32), np.asarray(b_v, np.float32),
        np.asarray(W_o, np.float32), np.asarray(b_o, np.float32),
    )
    if "nc" not in _CACHED:
        _CACHED["nc"] = build_nc()
    res = run_bass_kernel_spmd(
        _CACHED["nc"], in_maps, core_ids=list(range(N_CORES)), **run_kwargs
    )
    _CACHED["last_result"] = res
    full = np.empty((B, S, D), np.float32)
    for c in range(N_CORES):
        full[c // 4, S_OUT * (c % 4) : S_OUT * (c % 4 + 1), :] = res.results[c]["out"]
    return full


if __name__ == "__main__":
    rng = np.random.default_rng(0)
    build_nc()
    print("build ok")



# revision 6
# speedup vs baseline: 1.0752x; 1.0752x over previous
"""MultiHeadAttention forward on 8 Trainium2 NeuronCores (v2).

Problem: B=2, S=2048, D_MODEL=1024, H=16 heads, d_k=64, causal mask.

Sharding v2: core c -> (batch b = c//4, head-group hg = c%4). Each core
computes attention for heads {4hg..4hg+3} of batch b, so it only loads
batch b's activations (12 MB instead of 24 MB per core).

 - Projections: Q^T,K^T per head-pair p in {0,1} as [128, S] tiles
   (rows: head 2p d_k 0-63, head 2p+1 64-127); V as packed [128, 130]
   tiles per 128-kpos block (cols 0:64 head even | col 64 ones |
   65:129 head odd | col 129 ones) -- the ones columns produce softmax
   denominators inside the attn@V matmuls.
 - Attention per pair, q-blocks of 512, kt blocks of 128 kpos.
   ScoresT[kpos, q] via two row-group-concurrent matmuls (K=64 each).
   exp on ScalarE (scale 1/8 folded). Diagonal kt tiles restrict all
   work (scores / exp / attnV) to the live columns [128t, 512) and
   multiply only the [128,128] triangle block by a causal mask.
 - Normalization: evacuate av psum [65, 1024], reciprocal_approx_fast
   on the denominator row, gpsimd partition-broadcast, multiply into
   AFT (attn_flatT, bf16).
 - Output: per head-pair AllToAll over the 4 cores of the same batch
   redistributes AFT so core j gets d-rows {256i+128p} for its q-slice
   [512j, 512j+512). Pair-0's collective + half of the W_o matmul
   overlap pair-1's attention. out = lhs.T @ W_o + b_o.

Matmuls in bf16 (host casts); fp32 accumulation in PSUM.
"""

import sys

import numpy as np

sys.path.insert(0, "/opt/trn_rl_repo")

import ml_dtypes  # noqa: E402

import concourse.bacc as bacc  # noqa: E402
import concourse.mybir as mybir  # noqa: E402
import concourse.tile as tile  # noqa: E402
from concourse.bass_utils import run_bass_kernel_spmd  # noqa: E402

F32 = mybir.dt.float32
BF16 = mybir.dt.bfloat16
BF = ml_dtypes.bfloat16

B, S, D, H, DK = 2, 2048, 1024, 16, 64
N_CORES = 8
HPC = 4  # heads per core
DPC = HPC * DK  # 256 d_model cols per core
NKT = S // 128  # 16 kpos tiles
NQB = S // 512  # 4 q blocks

_CACHED = {}


def build_nc():
    nc = bacc.Bacc(num_devices=N_CORES)

    # ---- I/O (per core: batch b = c//4, cols 256*hg..) ----
    xq = nc.dram_tensor("xq", [D, S], BF16, kind="ExternalInput")  # q[b].T
    xk = nc.dram_tensor("xk", [D, S], BF16, kind="ExternalInput")
    xv = nc.dram_tensor("xv", [D, S], BF16, kind="ExternalInput")
    wq = nc.dram_tensor("wq", [D, DPC], BF16, kind="ExternalInput")
    wk = nc.dram_tensor("wk", [D, DPC], BF16, kind="ExternalInput")
    wv = nc.dram_tensor("wv", [D, DPC], BF16, kind="ExternalInput")
    wo = nc.dram_tensor("wo", [D, D], BF16, kind="ExternalInput")
    bq = nc.dram_tensor("bq", [128, 2], F32, kind="ExternalInput")
    bk = nc.dram_tensor("bk", [128, 2], F32, kind="ExternalInput")
    bv = nc.dram_tensor("bv", [1, DPC], F32, kind="ExternalInput")
    bo = nc.dram_tensor("bo", [1, D], F32, kind="ExternalInput")
    tri = nc.dram_tensor("tri", [128, 128], BF16, kind="ExternalInput")
    qoff = nc.dram_tensor("qoff", [1, 1], mybir.dt.int32, kind="ExternalInput")
    out = nc.dram_tensor("out", [512, D], F32, kind="ExternalOutput")

    with tile.TileContext(nc) as tc:
        with (
            tc.tile_pool(name="wtiles", bufs=1) as w_pool,
            tc.tile_pool(name="persist", bufs=1) as persist,
            tc.tile_pool(name="exp", bufs=4) as exp_pool,
            tc.tile_pool(name="small", bufs=2) as small_pool,
            tc.tile_pool(name="gen_ps", bufs=2, space="PSUM") as gen_ps,
            tc.tile_pool(name="score_ps", bufs=2, space="PSUM") as score_ps,
            tc.tile_pool(name="av_ps", bufs=1, space="PSUM") as av_ps,
            tc.tile_pool(name="dram", bufs=1, space="DRAM") as dram,
        ):
            # ---- persistent SBUF tensors ----
            QT = [persist.tile([128, S], BF16, tag=f"QT{p}", name=f"QT{p}") for p in range(2)]
            KT = [persist.tile([128, S], BF16, tag=f"KT{p}", name=f"KT{p}") for p in range(2)]
            VP = [
                [persist.tile([128, 130], BF16, tag=f"VP{p}_{i}", name=f"VP{p}_{i}") for i in range(NKT)]
                for p in range(2)
            ]
            AFT = [persist.tile([128, S], BF16, tag=f"AFT{p}", name=f"AFT{p}") for p in range(2)]
            tri_t = persist.tile([128, 128], BF16, tag="tri")
            nc.sync.dma_start(tri_t[:], tri[:])

            bq_t = persist.tile([128, 2], F32, tag="bq")
            bk_t = persist.tile([128, 2], F32, tag="bk")
            nc.sync.dma_start(bq_t[:], bq[:])
            nc.sync.dma_start(bk_t[:], bk[:])
            bv_bc = persist.tile([128, DPC], F32, tag="bvbc")
            nc.sync.dma_start(bv_bc[:], bv[:].partition_broadcast(128))
            bo_bc = persist.tile([128, D], F32, tag="bobc")
            nc.sync.dma_start(bo_bc[:], bo[:].partition_broadcast(128))

            wq_t = [w_pool.tile([128, DPC], BF16, tag=f"wq{d}", name=f"wq{d}") for d in range(8)]
            wk_t = [w_pool.tile([128, DPC], BF16, tag=f"wk{d}", name=f"wk{d}") for d in range(8)]
            wv_t = [w_pool.tile([128, DPC], BF16, tag=f"wv{d}", name=f"wv{d}") for d in range(8)]
            wo_t = [w_pool.tile([128, D], BF16, tag=f"wo{d}", name=f"wo{d}") for d in range(8)]
            for d in range(8):
                nc.sync.dma_start(wq_t[d][:], wq[128 * d : 128 * (d + 1), :])
                nc.sync.dma_start(wk_t[d][:], wk[128 * d : 128 * (d + 1), :])
                nc.sync.dma_start(wv_t[d][:], wv[128 * d : 128 * (d + 1), :])

            # ones columns of the packed V tiles (written once)
            for p in range(2):
                for i in range(NKT):
                    ones_ap = VP[p][i][:, 0:130].rearrange("a (h c) -> a h c", h=2)[:, :, 64:65]
                    nc.gpsimd.memset(ones_ap, 1.0)

            # ---- helpers ----
            def proj_qk(p, xt, wt, bias_t, dst):
                for sc in range(4):
                    ps = gen_ps.tile([128, 512], F32, tag="gen")
                    for d in range(8):
                        nc.tensor.matmul(
                            ps[:],
                            wt[d][:, 128 * p : 128 * (p + 1)],
                            xt[d][:, 512 * sc : 512 * (sc + 1)],
                            start=(d == 0),
                            stop=(d == 7),
                        )
                    nc.vector.tensor_scalar_add(
                        dst[:, 512 * sc : 512 * (sc + 1)], ps[:], bias_t[:, p : p + 1]
                    )

            def proj_v(xt):
                for ss in range(NKT):
                    ps = gen_ps.tile([128, 512], F32, tag="gen")
                    for d in range(8):
                        nc.tensor.matmul(
                            ps[:, 0:DPC],
                            xt[d][:, 128 * ss : 128 * (ss + 1)],
                            wv_t[d][:],
                            start=(d == 0),
                            stop=(d == 7),
                        )
                    for p in range(2):
                        dst = VP[p][ss][:, 0:130].rearrange("a (h c) -> a h c", h=2)[:, :, 0:64]
                        src = ps[:, 128 * p : 128 * (p + 1)].rearrange("a (h c) -> a h c", h=2)
                        bsl = bv_bc[:, 128 * p : 128 * (p + 1)].rearrange("a (h c) -> a h c", h=2)
                        nc.vector.tensor_add(dst, src, bsl)

            def attn_qblock(p, qb):
                qcol = 512 * qb
                n_kt = 4 * qb + 4
                av = av_ps.tile([65, 1024], F32, tag="av")

                def emit_scores(kt):
                    t = kt - 4 * qb
                    c0 = 128 * t if t >= 0 else 0
                    ps = score_ps.tile([128, 1024], F32, tag="sc", name="sc")
                    ksl = slice(128 * kt, 128 * (kt + 1))
                    for h in range(2):
                        nc.tensor.matmul(
                            ps[:, 512 * h + c0 : 512 * (h + 1)],
                            KT[p][64 * h : 64 * (h + 1), ksl],
                            QT[p][64 * h : 64 * (h + 1), qcol + c0 : qcol + 512],
                            start=True,
                            stop=True,
                        )
                    return ps

                ps_cur = emit_scores(0)
                for kt in range(n_kt):
                    t = kt - 4 * qb
                    c0 = 128 * t if t >= 0 else 0
                    et = exp_pool.tile([128, 1024], BF16, tag="et")
                    if c0 == 0:
                        nc.scalar.activation(
                            et[:], ps_cur[:], mybir.ActivationFunctionType.Exp, scale=0.125
                        )
                    else:
                        e3 = et[:, 0:1024].rearrange("a (h q) -> a h q", h=2)[:, :, c0:512]
                        p3 = ps_cur[:, 0:1024].rearrange("a (h q) -> a h q", h=2)[:, :, c0:512]
                        nc.scalar.activation(
                            e3, p3, mybir.ActivationFunctionType.Exp, scale=0.125
                        )
                    if kt + 1 < n_kt:
                        ps_cur = emit_scores(kt + 1)
                    if t >= 0:
                        for h in range(2):
                            dsl = slice(512 * h + c0, 512 * h + c0 + 128)
                            nc.vector.tensor_mul(et[:, dsl], et[:, dsl], tri_t[:])
                    for h in range(2):
                        nc.tensor.matmul(
                            av[:, 512 * h + c0 : 512 * (h + 1)],
                            VP[p][kt][:, 65 * h : 65 * (h + 1)],
                            et[:, 512 * h + c0 : 512 * (h + 1)],
                            start=(kt == 0),
                            stop=(kt == n_kt - 1),
                        )
                # normalize: evacuate psum, 1/denom, broadcast, scale into AFT
                avs = small_pool.tile([64, 1024], F32, tag="avs", name="avs")
                nc.vector.tensor_copy(avs[:], av[0:64, :])
                dn = small_pool.tile([1, 1024], F32, tag="dn")
                nc.vector.tensor_copy(dn[:], av[64:65, :])
                rcp = small_pool.tile([1, 1024], F32, tag="rcp")
                nc.vector.reciprocal_approx_fast(rcp[:], dn[:])
                rbc = small_pool.tile([64, 1024], F32, tag="rbc")
                nc.gpsimd.partition_broadcast(rbc[:], rcp[:])
                for h in range(2):
                    nc.vector.tensor_mul(
                        AFT[p][64 * h : 64 * (h + 1), qcol : qcol + 512],
                        avs[0:64, 512 * h : 512 * (h + 1)],
                        rbc[:, 512 * h : 512 * (h + 1)],
                    )

            # ---- load x, project, attention pair 0 (pair-1 proj interleaved) ----
            with (
                tc.tile_pool(name="xtq", bufs=8) as xq_pool,
                tc.tile_pool(name="xtk", bufs=8) as xk_pool,
                tc.tile_pool(name="xtv", bufs=8) as xv_pool,
            ):
                xk_t = [xk_pool.tile([128, S], BF16, tag="xtk", name="xtk") for _ in range(8)]
                xv_t = [xv_pool.tile([128, S], BF16, tag="xtv", name="xtv") for _ in range(8)]
                xq_t = [xq_pool.tile([128, S], BF16, tag="xtq", name="xtq") for _ in range(8)]
                for d in range(8):
                    dsl = slice(128 * d, 128 * (d + 1))
                    nc.sync.dma_start(xk_t[d][:], xk[dsl, :])
                for d in range(8):
                    dsl = slice(128 * d, 128 * (d + 1))
                    nc.sync.dma_start(xv_t[d][:], xv[dsl, :])
                for d in range(8):
                    dsl = slice(128 * d, 128 * (d + 1))
                    nc.sync.dma_start(xq_t[d][:], xq[dsl, :])

                proj_qk(0, xk_t, wk_t, bk_t, KT[0])
                proj_v(xv_t)
                proj_qk(0, xq_t, wq_t, bq_t, QT[0])

                # W_o loads after x so x DMAs get the early bandwidth
                for d in range(8):
                    nc.sync.dma_start(wo_t[d][:], wo[128 * d : 128 * (d + 1), :])

                attn_qblock(0, 0)
                proj_qk(1, xk_t, wk_t, bk_t, KT[1])
                attn_qblock(0, 1)
                proj_qk(1, xq_t, wq_t, bq_t, QT[1])
                attn_qblock(0, 2)
                attn_qblock(0, 3)

            with tc.tile_pool(name="late", bufs=1) as late:
                lhs_t = [
                    [late.tile([128, 512], BF16, tag=f"lhs{p}_{i}", name=f"lhs{p}_{i}") for i in range(4)]
                    for p in range(2)
                ]
                osb = [late.tile([128, D], F32, tag=f"osb{st}", name=f"osb{st}") for st in range(4)]

                a2a_in = [dram.tile([128, S], BF16, tag=f"a2a_in{p}", name=f"a2a_in{p}") for p in range(2)]
                a2a_out = [dram.tile([512, S], BF16, tag=f"a2a_out{p}", name=f"a2a_out{p}") for p in range(2)]

                # my q-block column offset (512 * (core % 4)), from a per-core input
                qoff_reg = nc.alloc_registers("qoff_reg")
                nc.regs_load(qoff_reg, qoff[0:1, 0:1])
                qoff_s = nc.snap(qoff_reg, donate=True, min_val=0, max_val=1536)

                def emit_a2a(p):
                    nc.sync.dma_start(a2a_in[p][:], AFT[p][:])
                    nc.gpsimd.collective_compute(
                        "AllGather",
                        mybir.AluOpType.bypass,
                        replica_groups=[[0, 1, 2, 3], [4, 5, 6, 7]],
                        ins=[a2a_in[p][:]],
                        outs=[a2a_out[p][:]],
                    )
                    import concourse.bass as bass_mod
                    for i in range(4):
                        nc.sync.dma_start(
                            lhs_t[p][i][:],
                            a2a_out[p][128 * i : 128 * (i + 1), bass_mod.ds(qoff_s, 512)],
                        )

                def emit_wo(p):
                    for st in range(4):
                        for nch in range(2):
                            ps = gen_ps.tile([128, 512], F32, tag="gen")
                            for i in range(4):
                                nc.tensor.matmul(
                                    ps[:],
                                    lhs_t[p][i][:, 128 * st : 128 * (st + 1)],
                                    wo_t[2 * i + p][:, 512 * nch : 512 * (nch + 1)],
                                    start=(i == 0),
                                    stop=(i == 3),
                                )
                            osl = slice(512 * nch, 512 * (nch + 1))
                            if p == 0:
                                nc.vector.tensor_add(osb[st][:, osl], ps[:], bo_bc[:, osl])
                            else:
                                nc.vector.tensor_add(osb[st][:, osl], ps[:], osb[st][:, osl])
                        if p == 1:
                            nc.sync.dma_start(out[128 * st : 128 * (st + 1), :], osb[st][:])

                emit_a2a(0)
                attn_qblock(1, 0)
                emit_wo(0)
                attn_qblock(1, 1)
                attn_qblock(1, 2)
                attn_qblock(1, 3)
                emit_a2a(1)
                emit_wo(1)

    nc.finalize()
    return nc


def _prep_in_maps(q, k, v, W_q, b_q, W_k, b_k, W_v, b_v, W_o, b_o):
    xT = [
        (
            np.ascontiguousarray(q[b].T.astype(BF)),
            np.ascontiguousarray(k[b].T.astype(BF)),
            np.ascontiguousarray(v[b].T.astype(BF)),
        )
        for b in range(B)
    ]
    wo_h = np.ascontiguousarray(W_o.astype(BF))
    bo_h = np.ascontiguousarray(b_o.reshape(1, D).astype(np.float32))
    i = np.arange(128)
    tri_h = np.ascontiguousarray((i[:, None] <= i[None, :]).astype(BF))

    in_maps = []
    for c in range(N_CORES):
        b, hg = c // 4, c % 4
        csl = slice(DPC * hg, DPC * (hg + 1))
        in_maps.append(
            {
                "xq": xT[b][0],
                "xk": xT[b][1],
                "xv": xT[b][2],
                "wq": np.ascontiguousarray(W_q[:, csl].astype(BF)),
                "wk": np.ascontiguousarray(W_k[:, csl].astype(BF)),
                "wv": np.ascontiguousarray(W_v[:, csl].astype(BF)),
                "wo": wo_h,
                "bq": np.ascontiguousarray(
                    b_q[csl].reshape(2, 128).T.astype(np.float32)
                ),
                "bk": np.ascontiguousarray(
                    b_k[csl].reshape(2, 128).T.astype(np.float32)
                ),
                "bv": np.ascontiguousarray(b_v[csl].reshape(1, DPC).astype(np.float32)),
                "bo": bo_h,
                "tri": tri_h,
                "qoff": np.array([[512 * hg]], dtype=np.int32),
            }
        )
    return in_maps


def kernel(q, k, v, mask, W_q, b_q, W_k, b_k, W_v, b_v, W_o, b_o, **run_kwargs):
    q, k, v = (np.asarray(t, np.float32) for t in (q, k, v))
    in_maps = _prep_in_maps(
        q, k, v,
        np.asarray(W_q, np.float32), np.asarray(b_q, np.float32),
        np.asarray(W_k, np.float32), np.asarray(b_k, np.float32),
        np.asarray(W_v, np.float32), np.asarray(b_v, np.float32),
        np.asarray(W_o, np.float32), np.asarray(b_o, np.float32),
    )
    if "nc" not in _CACHED:
        _CACHED["nc"] = build_nc()
    res = run_bass_kernel_spmd(
        _CACHED["nc"], in_maps, core_ids=list(range(N_CORES)), **run_kwargs
    )
    _CACHED["last_result"] = res
    full = np.empty((B, S, D), np.float32)
    for c in range(N_CORES):
        b, hg = c // 4, c % 4
        full[b, 512 * hg : 512 * (hg + 1), :] = res.results[c]["out"]
    return full


if __name__ == "__main__":
    build_nc()
    print("build ok")


# revision 7
# speedup vs baseline: 1.2084x; 1.1239x over previous
"""MultiHeadAttention forward on 8 Trainium2 NeuronCores (v2).

Problem: B=2, S=2048, D_MODEL=1024, H=16 heads, d_k=64, causal mask.

Sharding v2: core c -> (batch b = c//4, head-group hg = c%4). Each core
computes attention for heads {4hg..4hg+3} of batch b, so it only loads
batch b's activations (12 MB instead of 24 MB per core).

 - Projections: Q^T,K^T per head-pair p in {0,1} as [128, S] tiles
   (rows: head 2p d_k 0-63, head 2p+1 64-127); V as packed [128, 130]
   tiles per 128-kpos block (cols 0:64 head even | col 64 ones |
   65:129 head odd | col 129 ones) -- the ones columns produce softmax
   denominators inside the attn@V matmuls.
 - Attention per pair, q-blocks of 512, kt blocks of 128 kpos.
   ScoresT[kpos, q] via two row-group-concurrent matmuls (K=64 each).
   exp on ScalarE (scale 1/8 folded). Diagonal kt tiles restrict all
   work (scores / exp / attnV) to the live columns [128t, 512) and
   multiply only the [128,128] triangle block by a causal mask.
 - Normalization: evacuate av psum [65, 1024], reciprocal_approx_fast
   on the denominator row, gpsimd partition-broadcast, multiply into
   AFT (attn_flatT, bf16).
 - Output: per head-pair AllToAll over the 4 cores of the same batch
   redistributes AFT so core j gets d-rows {256i+128p} for its q-slice
   [512j, 512j+512). Pair-0's collective + half of the W_o matmul
   overlap pair-1's attention. out = lhs.T @ W_o + b_o.

Matmuls in bf16 (host casts); fp32 accumulation in PSUM.
"""

import sys

import numpy as np

sys.path.insert(0, "/opt/trn_rl_repo")

import ml_dtypes  # noqa: E402

import concourse.bacc as bacc  # noqa: E402
import concourse.mybir as mybir  # noqa: E402
import concourse.tile as tile  # noqa: E402
from concourse.bass_utils import run_bass_kernel_spmd  # noqa: E402

F32 = mybir.dt.float32
BF16 = mybir.dt.bfloat16
BF = ml_dtypes.bfloat16

B, S, D, H, DK = 2, 2048, 1024, 16, 64
N_CORES = 8
HPC = 4  # heads per core
DPC = HPC * DK  # 256 d_model cols per core
NKT = S // 128  # 16 kpos tiles
NQB = S // 512  # 4 q blocks

_CACHED = {}


def build_nc():
    nc = bacc.Bacc(num_devices=N_CORES)

    # ---- I/O (per core: batch b = c//4, cols 256*hg..) ----
    xq = nc.dram_tensor("xq", [D, S], BF16, kind="ExternalInput")  # q[b].T
    xk = nc.dram_tensor("xk", [D, S], BF16, kind="ExternalInput")
    xv = nc.dram_tensor("xv", [D, S], BF16, kind="ExternalInput")
    wq = nc.dram_tensor("wq", [D, DPC], BF16, kind="ExternalInput")
    wk = nc.dram_tensor("wk", [D, DPC], BF16, kind="ExternalInput")
    wv = nc.dram_tensor("wv", [D, DPC], BF16, kind="ExternalInput")
    wo = nc.dram_tensor("wo", [D, D], BF16, kind="ExternalInput")
    bq = nc.dram_tensor("bq", [128, 2], F32, kind="ExternalInput")
    bk = nc.dram_tensor("bk", [128, 2], F32, kind="ExternalInput")
    bv = nc.dram_tensor("bv", [1, DPC], F32, kind="ExternalInput")
    bo = nc.dram_tensor("bo", [1, D], F32, kind="ExternalInput")
    tri = nc.dram_tensor("tri", [128, 128], BF16, kind="ExternalInput")
    qoff = nc.dram_tensor("qoff", [1, 1], mybir.dt.int32, kind="ExternalInput")
    out = nc.dram_tensor("out", [512, D], F32, kind="ExternalOutput")

    with tile.TileContext(nc) as tc:
        with (
            tc.tile_pool(name="wtiles", bufs=1) as w_pool,
            tc.tile_pool(name="persist", bufs=1) as persist,
            tc.tile_pool(name="exp", bufs=4) as exp_pool,
            tc.tile_pool(name="small", bufs=2) as small_pool,
            tc.tile_pool(name="gen_ps", bufs=2, space="PSUM") as gen_ps,
            tc.tile_pool(name="score_ps", bufs=2, space="PSUM") as score_ps,
            tc.tile_pool(name="av_ps", bufs=1, space="PSUM") as av_ps,
            tc.tile_pool(name="dram", bufs=1, space="DRAM") as dram,
        ):
            # ---- persistent SBUF tensors ----
            QT = [persist.tile([128, S], BF16, tag=f"QT{p}", name=f"QT{p}") for p in range(2)]
            KT = [persist.tile([128, S], BF16, tag=f"KT{p}", name=f"KT{p}") for p in range(2)]
            VP = [
                [persist.tile([128, 130], BF16, tag=f"VP{p}_{i}", name=f"VP{p}_{i}") for i in range(NKT)]
                for p in range(2)
            ]
            AFT = [persist.tile([128, S], BF16, tag=f"AFT{p}", name=f"AFT{p}") for p in range(2)]
            tri_t = persist.tile([128, 128], BF16, tag="tri")
            nc.sync.dma_start(tri_t[:], tri[:])

            bq_t = persist.tile([128, 2], F32, tag="bq")
            bk_t = persist.tile([128, 2], F32, tag="bk")
            nc.sync.dma_start(bq_t[:], bq[:])
            nc.sync.dma_start(bk_t[:], bk[:])
            bv_bc = persist.tile([128, DPC], F32, tag="bvbc")
            nc.sync.dma_start(bv_bc[:], bv[:].partition_broadcast(128))
            bo_bc = persist.tile([128, D], F32, tag="bobc")
            nc.sync.dma_start(bo_bc[:], bo[:].partition_broadcast(128))

            wq_t = [w_pool.tile([128, DPC], BF16, tag=f"wq{d}", name=f"wq{d}") for d in range(8)]
            wk_t = [w_pool.tile([128, DPC], BF16, tag=f"wk{d}", name=f"wk{d}") for d in range(8)]
            wv_t = [w_pool.tile([128, DPC], BF16, tag=f"wv{d}", name=f"wv{d}") for d in range(8)]
            wo_t = [w_pool.tile([128, D], BF16, tag=f"wo{d}", name=f"wo{d}") for d in range(8)]
            for d in range(8):
                nc.sync.dma_start(wq_t[d][:], wq[128 * d : 128 * (d + 1), :])
                nc.sync.dma_start(wk_t[d][:], wk[128 * d : 128 * (d + 1), :])
                nc.sync.dma_start(wv_t[d][:], wv[128 * d : 128 * (d + 1), :])

            # ones columns of the packed V tiles (written once)
            for p in range(2):
                for i in range(NKT):
                    ones_ap = VP[p][i][:, 0:130].rearrange("a (h c) -> a h c", h=2)[:, :, 64:65]
                    nc.gpsimd.memset(ones_ap, 1.0)

            # ---- helpers ----
            def proj_qk_sc(p, sc, xt, wt, bias_t, dst):
                ps = gen_ps.tile([128, 512], F32, tag="gen", name="gen")
                for d in range(8):
                    nc.tensor.matmul(
                        ps[:],
                        wt[d][:, 128 * p : 128 * (p + 1)],
                        xt[d][:, 512 * sc : 512 * (sc + 1)],
                        start=(d == 0),
                        stop=(d == 7),
                    )
                nc.vector.tensor_scalar_add(
                    dst[:, 512 * sc : 512 * (sc + 1)], ps[:], bias_t[:, p : p + 1]
                )

            def proj_qk(p, xt, wt, bias_t, dst):
                for sc in range(4):
                    proj_qk_sc(p, sc, xt, wt, bias_t, dst)

            def proj_v_sc(sc, xt):
                for ss in range(4 * sc, 4 * sc + 4):
                    ps = gen_ps.tile([128, 512], F32, tag="gen", name="gen")
                    for d in range(8):
                        nc.tensor.matmul(
                            ps[:, 0:DPC],
                            xt[d][:, 128 * ss : 128 * (ss + 1)],
                            wv_t[d][:],
                            start=(d == 0),
                            stop=(d == 7),
                        )
                    for p in range(2):
                        dst = VP[p][ss][:, 0:130].rearrange("a (h c) -> a h c", h=2)[:, :, 0:64]
                        src = ps[:, 128 * p : 128 * (p + 1)].rearrange("a (h c) -> a h c", h=2)
                        bsl = bv_bc[:, 128 * p : 128 * (p + 1)].rearrange("a (h c) -> a h c", h=2)
                        nc.vector.tensor_add(dst, src, bsl)

            def attn_qblock(p, qb):
                qcol = 512 * qb
                n_kt = 4 * qb + 4
                av = av_ps.tile([65, 1024], F32, tag="av")

                def emit_scores(kt):
                    t = kt - 4 * qb
                    c0 = 128 * t if t >= 0 else 0
                    ps = score_ps.tile([128, 1024], F32, tag="sc", name="sc")
                    ksl = slice(128 * kt, 128 * (kt + 1))
                    for h in range(2):
                        nc.tensor.matmul(
                            ps[:, 512 * h + c0 : 512 * (h + 1)],
                            KT[p][64 * h : 64 * (h + 1), ksl],
                            QT[p][64 * h : 64 * (h + 1), qcol + c0 : qcol + 512],
                            start=True,
                            stop=True,
                        )
                    return ps

                ps_cur = emit_scores(0)
                for kt in range(n_kt):
                    t = kt - 4 * qb
                    c0 = 128 * t if t >= 0 else 0
                    et = exp_pool.tile([128, 1024], BF16, tag="et")
                    if c0 == 0:
                        nc.scalar.activation(
                            et[:], ps_cur[:], mybir.ActivationFunctionType.Exp, scale=0.125
                        )
                    else:
                        e3 = et[:, 0:1024].rearrange("a (h q) -> a h q", h=2)[:, :, c0:512]
                        p3 = ps_cur[:, 0:1024].rearrange("a (h q) -> a h q", h=2)[:, :, c0:512]
                        nc.scalar.activation(
                            e3, p3, mybir.ActivationFunctionType.Exp, scale=0.125
                        )
                    if kt + 1 < n_kt:
                        ps_cur = emit_scores(kt + 1)
                    if t >= 0:
                        for h in range(2):
                            dsl = slice(512 * h + c0, 512 * h + c0 + 128)
                            nc.vector.tensor_mul(et[:, dsl], et[:, dsl], tri_t[:])
                    for h in range(2):
                        nc.tensor.matmul(
                            av[:, 512 * h + c0 : 512 * (h + 1)],
                            VP[p][kt][:, 65 * h : 65 * (h + 1)],
                            et[:, 512 * h + c0 : 512 * (h + 1)],
                            start=(kt == 0),
                            stop=(kt == n_kt - 1),
                        )
                # normalize: evacuate psum, 1/denom, broadcast, scale into AFT
                avs = small_pool.tile([64, 1024], F32, tag="avs", name="avs")
                nc.vector.tensor_copy(avs[:], av[0:64, :])
                dn = small_pool.tile([1, 1024], F32, tag="dn")
                nc.vector.tensor_copy(dn[:], av[64:65, :])
                rcp = small_pool.tile([1, 1024], F32, tag="rcp")
                nc.vector.reciprocal_approx_fast(rcp[:], dn[:])
                rbc = small_pool.tile([64, 1024], F32, tag="rbc")
                nc.gpsimd.partition_broadcast(rbc[:], rcp[:])
                for h in range(2):
                    nc.vector.tensor_mul(
                        AFT[p][64 * h : 64 * (h + 1), qcol : qcol + 512],
                        avs[0:64, 512 * h : 512 * (h + 1)],
                        rbc[:, 512 * h : 512 * (h + 1)],
                    )

            # ---- load x, project, attention pair 0 (pair-1 proj interleaved) ----
            with (
                tc.tile_pool(name="xtq", bufs=8) as xq_pool,
                tc.tile_pool(name="xtk", bufs=8) as xk_pool,
                tc.tile_pool(name="xtv", bufs=8) as xv_pool,
            ):
                xk_t = [xk_pool.tile([128, S], BF16, tag="xtk", name="xtk") for _ in range(8)]
                xv_t = [xv_pool.tile([128, S], BF16, tag="xtv", name="xtv") for _ in range(8)]
                xq_t = [xq_pool.tile([128, S], BF16, tag="xtq", name="xtq") for _ in range(8)]
                # chunked loads (512 cols at a time) so pair-0 projections and
                # attention q-block 0 start as soon as the first chunks land
                for sc in range(4):
                    ssl = slice(512 * sc, 512 * (sc + 1))
                    for xt_, xd in ((xk_t, xk), (xv_t, xv), (xq_t, xq)):
                        for d in range(8):
                            nc.sync.dma_start(
                                xt_[d][:, ssl], xd[128 * d : 128 * (d + 1), ssl]
                            )
                    proj_qk_sc(0, sc, xk_t, wk_t, bk_t, KT[0])
                    proj_v_sc(sc, xv_t)
                    proj_qk_sc(0, sc, xq_t, wq_t, bq_t, QT[0])

                # W_o loads after x so x DMAs get the early bandwidth
                for d in range(8):
                    nc.sync.dma_start(wo_t[d][:], wo[128 * d : 128 * (d + 1), :])

                attn_qblock(0, 0)
                proj_qk(1, xk_t, wk_t, bk_t, KT[1])
                attn_qblock(0, 1)
                proj_qk(1, xq_t, wq_t, bq_t, QT[1])
                attn_qblock(0, 2)
                attn_qblock(0, 3)

            with tc.tile_pool(name="late", bufs=1) as late:
                lhs_t = [
                    [late.tile([128, 512], BF16, tag=f"lhs{p}_{i}", name=f"lhs{p}_{i}") for i in range(4)]
                    for p in range(2)
                ]
                osb = [late.tile([128, D], F32, tag=f"osb{st}", name=f"osb{st}") for st in range(4)]

                a2a_in = [dram.tile([1024, 512], BF16, tag=f"a2a_in{p}", name=f"a2a_in{p}") for p in range(2)]
                a2a_out = [dram.tile([2, 512, 512], BF16, tag=f"a2a_out{p}", name=f"a2a_out{p}") for p in range(2)]

                # my batch-group (core // 4), from a per-core input
                import concourse.bass as bass_mod
                gsel_reg = nc.alloc_registers("gsel_reg")
                nc.regs_load(gsel_reg, qoff[0:1, 0:1])
                gsel_s = nc.snap(gsel_reg, donate=True, min_val=0, max_val=1)

                def emit_a2a(p):
                    # 8-core AllToAll; shard for dest j = my AFT cols of dest's
                    # q-block (同 data duplicated for the other batch group)
                    for j in range(8):
                        nc.sync.dma_start(
                            a2a_in[p][128 * j : 128 * (j + 1), :],
                            AFT[p][:, 512 * (j % 4) : 512 * (j % 4 + 1)],
                        )
                    nc.gpsimd.collective_compute(
                        "AllToAll",
                        mybir.AluOpType.bypass,
                        replica_groups=[list(range(8))],
                        ins=[a2a_in[p][:]],
                        outs=[a2a_out[p][:]],
                    )
                    for i in range(4):
                        nc.sync.dma_start(
                            lhs_t[p][i][:],
                            a2a_out[p][bass_mod.ds(gsel_s, 1), 128 * i : 128 * (i + 1), :],
                        )

                def emit_wo(p):
                    for st in range(4):
                        for nch in range(2):
                            ps = gen_ps.tile([128, 512], F32, tag="gen")
                            for i in range(4):
                                nc.tensor.matmul(
                                    ps[:],
                                    lhs_t[p][i][:, 128 * st : 128 * (st + 1)],
                                    wo_t[2 * i + p][:, 512 * nch : 512 * (nch + 1)],
                                    start=(i == 0),
                                    stop=(i == 3),
                                )
                            osl = slice(512 * nch, 512 * (nch + 1))
                            if p == 0:
                                nc.vector.tensor_add(osb[st][:, osl], ps[:], bo_bc[:, osl])
                            else:
                                nc.vector.tensor_add(osb[st][:, osl], ps[:], osb[st][:, osl])
                        if p == 1:
                            nc.sync.dma_start(out[128 * st : 128 * (st + 1), :], osb[st][:])

                emit_a2a(0)
                attn_qblock(1, 0)
                attn_qblock(1, 1)
                attn_qblock(1, 2)
                attn_qblock(1, 3)
                emit_a2a(1)
                emit_wo(0)
                emit_wo(1)

    nc.finalize()
    return nc


def _prep_in_maps(q, k, v, W_q, b_q, W_k, b_k, W_v, b_v, W_o, b_o):
    xT = [
        (
            np.ascontiguousarray(q[b].T.astype(BF)),
            np.ascontiguousarray(k[b].T.astype(BF)),
            np.ascontiguousarray(v[b].T.astype(BF)),
        )
        for b in range(B)
    ]
    wo_h = np.ascontiguousarray(W_o.astype(BF))
    bo_h = np.ascontiguousarray(b_o.reshape(1, D).astype(np.float32))
    i = np.arange(128)
    tri_h = np.ascontiguousarray((i[:, None] <= i[None, :]).astype(BF))

    in_maps = []
    for c in range(N_CORES):
        b, hg = c // 4, c % 4
        csl = slice(DPC * hg, DPC * (hg + 1))
        in_maps.append(
            {
                "xq": xT[b][0],
                "xk": xT[b][1],
                "xv": xT[b][2],
                "wq": np.ascontiguousarray(W_q[:, csl].astype(BF)),
                "wk": np.ascontiguousarray(W_k[:, csl].astype(BF)),
                "wv": np.ascontiguousarray(W_v[:, csl].astype(BF)),
                "wo": wo_h,
                "bq": np.ascontiguousarray(
                    b_q[csl].reshape(2, 128).T.astype(np.float32)
                ),
                "bk": np.ascontiguousarray(
                    b_k[csl].reshape(2, 128).T.astype(np.float32)
                ),
                "bv": np.ascontiguousarray(b_v[csl].reshape(1, DPC).astype(np.float32)),
                "bo": bo_h,
                "tri": tri_h,
                "qoff": np.array([[b]], dtype=np.int32),
            }
        )
    return in_maps


def kernel(q, k, v, mask, W_q, b_q, W_k, b_k, W_v, b_v, W_o, b_o, **run_kwargs):
    q, k, v = (np.asarray(t, np.float32) for t in (q, k, v))
    in_maps = _prep_in_maps(
        q, k, v,
        np.asarray(W_q, np.float32), np.asarray(b_q, np.float32),
        np.asarray(W_k, np.float32), np.asarray(b_k, np.float32),
        np.asarray(W_v, np.float32), np.asarray(b_v, np.float32),
        np.asarray(W_o, np.float32), np.asarray(b_o, np.float32),
    )
    if "nc" not in _CACHED:
        _CACHED["nc"] = build_nc()
    res = run_bass_kernel_spmd(
        _CACHED["nc"], in_maps, core_ids=list(range(N_CORES)), **run_kwargs
    )
    _CACHED["last_result"] = res
    full = np.empty((B, S, D), np.float32)
    for c in range(N_CORES):
        b, hg = c // 4, c % 4
        full[b, 512 * hg : 512 * (hg + 1), :] = res.results[c]["out"]
    return full


if __name__ == "__main__":
    build_nc()
    print("build ok")
